# revision 1
# baseline (speedup 1.0000x reference)
"""Trainium2 Bass kernel for nn_Coords2RMSD (masked Kabsch RMSD loss).

Pure data parallel over 8 NeuronCores (1024 samples each). Inputs are
pre-planarized on the host (each row [x1(768)|x2(768)|x3(768)]) so every
device op is contiguous. Per core, samples are processed in 8 tiles of
128 (partition = sample). Each tile's X/Y rows stream from HBM once; 17
per-sample reductions (mask-weighted component sums, sums of squares, and
the 3x3 correlation matrix) are computed with fused multiply-accumulate
ops spread across DVE / GPSIMD / ACT. A closed-form 3x3 eigenvalue
epilogue (trig method; polynomial acos/sin/cos; sqrt via exp(0.5*ln))
turns the reductions into the RMSD.
"""
import math
import numpy as np

P = 128          # partitions (samples per tile)
M = 768          # max atoms
D = 3 * M        # row length
NCORES = 8
T = 8            # tiles per core
S = P * T        # samples per core
YM_DVE = 1536    # elements of ym built on DVE (rest on gpsimd)

_CACHE = {}


def _build(n_tiles):
    import concourse.bacc as bacc
    import concourse.mybir as mybir
    from concourse.tile import TileContext
    from concourse.hw_specs import get_activation_tables

    f32 = mybir.dt.float32
    bf16 = mybir.dt.bfloat16
    ALU = mybir.AluOpType
    AF = mybir.ActivationFunctionType

    Tn = n_tiles
    Sn = P * Tn

    nc = bacc.Bacc()
    xd = nc.declare_dram_parameter("x", [Sn, D], f32, isOutput=False)
    yd = nc.declare_dram_parameter("y", [Sn, D], f32, isOutput=False)
    # consts packs [iota_planar (D) | nv (Tn) | invn (Tn)]
    constsd = nc.declare_dram_parameter("consts", [P, D + 2 * Tn], f32,
                                        isOutput=False)
    outd = nc.declare_dram_parameter("out", [P, Tn], f32, isOutput=True)

    with TileContext(nc) as tc:
        with tc.tile_pool(name="io", bufs=3) as io, \
             tc.tile_pool(name="wk", bufs=2) as wk, \
             tc.tile_pool(name="st", bufs=1) as st:
            consts_t = st.tile([P, D + 2 * Tn], f32)
            nc.sync.dma_start(out=consts_t[:], in_=constsd[:])
            iota_t = consts_t[:, 0:D]           # planar atom index (x3)
            nv_t = consts_t[:, D:D + Tn]
            invn_t = consts_t[:, D + Tn:D + 2 * Tn]

            # stats accumulators
            mm = st.tile([P, 9 * Tn], f32)    # col (i*3+j)*Tn + t
            sx = st.tile([P, 3 * Tn], f32)    # col i*Tn + t
            sy = st.tile([P, 3 * Tn], f32)
            ssx = st.tile([P, Tn], f32)
            ssy = st.tile([P, Tn], f32)

            for t in range(Tn):
                xt = io.tile([P, D], f32, tag="x")
                nc.sync.dma_start(out=xt[:], in_=xd[t * P:(t + 1) * P, :])
                yt = io.tile([P, D], f32, tag="y")
                nc.sync.dma_start(out=yt[:], in_=yd[t * P:(t + 1) * P, :])

                # mask = (iota < n), planar, on DVE (single-src 2x mode)
                mask3 = wk.tile([P, D], f32, tag="mask3")
                nc.vector.tensor_scalar(out=mask3[:], in0=iota_t,
                                        scalar1=nv_t[:, t:t + 1], scalar2=None,
                                        op0=ALU.is_lt)

                # masked tensors in bf16 (cast on write): xm + ym head on
                # gpsimd, ym tail on DVE
                xm = wk.tile([P, D], bf16, tag="xm")
                nc.gpsimd.tensor_tensor(out=xm[:], in0=xt[:], in1=mask3[:],
                                        op=ALU.mult)
                ym = wk.tile([P, D], bf16, tag="ym")
                h = D - YM_DVE
                nc.gpsimd.tensor_tensor(out=ym[:, :h], in0=yt[:, :h],
                                        in1=mask3[:, :h], op=ALU.mult)
                nc.vector.tensor_tensor(out=ym[:, h:], in0=yt[:, h:],
                                        in1=mask3[:, h:], op=ALU.mult)

                # products: m_ij = sum_a xm_i * ym_j (fused accumulate, DVE,
                # bf16 inputs for the 2x perf mode; fp32 accumulator)
                for i in range(3):
                    for j in range(3):
                        junk = wk.tile([P, M], bf16, tag="junk")
                        col = (i * 3 + j) * Tn + t
                        nc.vector.scalar_tensor_tensor(
                            out=junk[:], in0=xm[:, i * M:(i + 1) * M],
                            scalar=1.0, in1=ym[:, j * M:(j + 1) * M],
                            op0=ALU.mult, op1=ALU.mult,
                            accum_out=mm[:, col:col + 1])

                # X sumsq on DVE (bf16 2x), Y sumsq on ACT, comp sums on ACT
                junk2 = wk.tile([P, D], bf16, tag="junk2")
                nc.vector.scalar_tensor_tensor(
                    out=junk2[:], in0=xm[:], scalar=1.0, in1=xm[:],
                    op0=ALU.mult, op1=ALU.mult,
                    accum_out=ssx[:, t:t + 1])
                sq2 = wk.tile([P, D], bf16, tag="sq")
                nc.scalar.activation(out=sq2[:], in_=ym[:], func=AF.Square,
                                     accum_out=ssy[:, t:t + 1])
                for i in range(3):
                    cp2 = wk.tile([P, M], bf16, tag="cp")
                    nc.scalar.activation(out=cp2[:], in_=ym[:, i * M:(i + 1) * M],
                                         func=AF.Copy,
                                         accum_out=sy[:, i * Tn + t:i * Tn + t + 1])
                    cp = wk.tile([P, M], bf16, tag="cp")
                    nc.scalar.activation(out=cp[:], in_=xm[:, i * M:(i + 1) * M],
                                         func=AF.Copy,
                                         accum_out=sx[:, i * Tn + t:i * Tn + t + 1])

            # ---------------- epilogue (batched over [P, ..., Tn]) ----------
            cnt = [0]

            def new(shape):
                """Allocate a scratch tile; return an AP shaped like `shape`."""
                cnt[0] += 1
                free = int(np.prod(shape[1:]))
                r = st.tile([P, free], f32, tag=f"e{cnt[0]}")
                ap = r[:]
                if len(shape) > 2:
                    names = " ".join(f"d{i}" for i in range(len(shape) - 1))
                    ap = ap.rearrange(f"p ({names}) -> p {names}",
                                      **{f"d{i}": int(shape[1 + i])
                                         for i in range(len(shape) - 1)})
                return ap

            def tt(a, b, op, shape=None):
                r = new(list(shape or a.shape))
                nc.vector.tensor_tensor(out=r, in0=a, in1=b, op=op)
                return r

            def ts(a, s1, op0, s2=None, op1=None):
                r = new(list(a.shape))
                if op1 is None:
                    nc.vector.tensor_scalar(out=r, in0=a, scalar1=s1,
                                            scalar2=None, op0=op0)
                else:
                    nc.vector.tensor_scalar(out=r, in0=a, scalar1=s1,
                                            scalar2=s2, op0=op0, op1=op1)
                return r

            def stt(a, s, b, op0, op1):
                r = new(list(a.shape))
                nc.vector.scalar_tensor_tensor(out=r, in0=a, scalar=s,
                                               in1=b, op0=op0, op1=op1)
                return r

            def act(a, func, scale=1.0, bias=0.0):
                r = new(list(a.shape))
                nc.scalar.activation(out=r, in_=a, func=func,
                                     scale=scale, bias=bias)
                return r

            def recip(a):
                r = new(list(a.shape))
                nc.vector.reciprocal(out=r, in_=a)
                return r

            def red_inner(a, n_keep):
                r = new([P, n_keep])
                nc.vector.tensor_reduce(out=r, in_=a,
                                        axis=mybir.AxisListType.X, op=ALU.add)
                return r

            def poly_eval(x, coeffs):
                """coeffs [a_n..a_1, a_0] -> a_0 + x*(a_1 + x*(...a_n))"""
                g = ts(x, coeffs[0], ALU.mult)
                for c in coeffs[1:-1]:
                    g = stt(g, c, x, ALU.add, ALU.mult)
                return ts(g, coeffs[-1], ALU.add)

            mmv = mm[:].rearrange("p (i j t) -> p i j t", i=3, j=3)
            sxv = sx[:].rearrange("p (i t) -> p i t", i=3)
            syv = sy[:].rearrange("p (i t) -> p i t", i=3)
            invn_b3 = invn_t.unsqueeze(1).broadcast_to([P, 3, Tn])

            # R_ij = m_ij - (sx_i * invn) * sy_j
            meanx = tt(sxv, invn_b3, ALU.mult)                       # [P,3,Tn]
            meanx_v = meanx.unsqueeze(2).broadcast_to([P, 3, 3, Tn])
            sy_v = syv.unsqueeze(1).broadcast_to([P, 3, 3, Tn])
            mxsy = tt(meanx_v, sy_v, ALU.mult)
            Rv = tt(mmv, mxsy, ALU.subtract)                         # [P,3,3,Tn]

            # e0 = ssx + ssy - (|sx|^2 + |sy|^2) * invn
            sx2 = tt(sxv, sxv, ALU.mult)
            sy2 = tt(syv, syv, ALU.mult)
            nrm = tt(sx2, sy2, ALU.add)
            nrms = red_inner(nrm.rearrange("p i t -> p t i"), Tn)
            ss = tt(ssx[:], ssy[:], ALU.add)
            nrmi = tt(nrms, invn_t, ALU.mult)
            e0 = tt(ss, nrmi, ALU.subtract)                          # [P,Tn]

            # A = R^T R (batched outer products over k)
            Av = new([P, 3, 3, Tn])
            for k in range(3):
                rk = Rv[:, k]
                rk_a = rk.unsqueeze(2).broadcast_to([P, 3, 3, Tn])
                rk_b = rk.unsqueeze(1).broadcast_to([P, 3, 3, Tn])
                if k == 0:
                    nc.vector.tensor_tensor(out=Av, in0=rk_a, in1=rk_b,
                                            op=ALU.mult)
                else:
                    pk = tt(rk_a, rk_b, ALU.mult)
                    nc.vector.tensor_tensor(out=Av, in0=Av, in1=pk, op=ALU.add)
            Aflat = Av.rearrange("p a b t -> p (a b) t")
            Adiag = Aflat[:, ::4]                                    # [P,3,Tn]

            q = ts(red_inner(Adiag.rearrange("p a t -> p t a"), Tn),
                   1.0 / 3.0, ALU.mult)                              # [P,Tn]
            q_b3 = q.unsqueeze(1).broadcast_to([P, 3, Tn])
            bdiag = tt(Adiag, q_b3, ALU.subtract)

            # p2 = sum(bdiag^2) + (sum(A^2) - sum(diag(A)^2))
            asq = tt(Aflat, Aflat, ALU.mult)
            allsq = red_inner(asq.rearrange("p a t -> p t a"), Tn)
            dsq = tt(Adiag, Adiag, ALU.mult)
            dsqs = red_inner(dsq.rearrange("p a t -> p t a"), Tn)
            bsq = tt(bdiag, bdiag, ALU.mult)
            bsqs = red_inner(bsq.rearrange("p a t -> p t a"), Tn)
            offs = tt(allsq, dsqs, ALU.subtract)
            p2 = tt(bsqs, offs, ALU.add)                             # [P,Tn]

            # log-space: p = (p2/6)^0.5 and invp^3 = (p2/6)^-1.5
            p2e = ts(p2, 1e-10, ALU.add)
            lnp2 = act(p2e, AF.Ln, scale=1.0 / 6.0)
            p_ = act(lnp2, AF.Exp, scale=0.5)
            ip3 = act(lnp2, AF.Exp, scale=-1.5)

            # batched determinants of W0=R and W1=B (= A - q I)
            Dw = new([P, 2, 3, 3, Tn])
            nc.vector.tensor_copy(Dw[:, 0], Rv)
            nc.vector.tensor_copy(Dw[:, 1], Av)
            Dw_diag = Dw.rearrange("p w a b t -> p w (a b) t")[:, 1, ::4]
            nc.vector.tensor_tensor(out=Dw_diag, in0=Adiag, in1=q_b3,
                                    op=ALU.subtract)

            def dsl(i, j):
                return Dw[:, :, i, j]                                # [P,2,Tn]

            u1 = tt(dsl(1, 1), dsl(2, 2), ALU.mult)
            u2 = tt(dsl(1, 2), dsl(2, 1), ALU.mult)
            cof0 = tt(dsl(0, 0), tt(u1, u2, ALU.subtract), ALU.mult)
            u3 = tt(dsl(1, 0), dsl(2, 2), ALU.mult)
            u4 = tt(dsl(1, 2), dsl(2, 0), ALU.mult)
            cof1 = tt(dsl(0, 1), tt(u3, u4, ALU.subtract), ALU.mult)
            u5 = tt(dsl(1, 0), dsl(2, 1), ALU.mult)
            u6 = tt(dsl(1, 1), dsl(2, 0), ALU.mult)
            cof2 = tt(dsl(0, 2), tt(u5, u6, ALU.subtract), ALU.mult)
            dets = tt(tt(cof0, cof1, ALU.subtract), cof2, ALU.add)   # [P,2,Tn]
            detR = dets[:, 0]
            detB = dets[:, 1]

            # r = clamp(0.5 * detB * invp^3, -1, 1)
            rr = tt(detB, ip3, ALU.mult)
            r_ = ts(rr, 0.5, ALU.mult, 1.0, ALU.min)
            r_ = ts(r_, -1.0, ALU.max)

            # acos(r)/3 via |r| polynomial (A&S 4.4.46) + reflection
            rneg = ts(r_, -1.0, ALU.mult)
            tabs = tt(r_, rneg, ALU.max)
            poly = poly_eval(tabs, [-0.0012624911, 0.0066700901, -0.0170881256,
                                    0.0308918810, -0.0501743046, 0.0889789874,
                                    -0.2145988016, 1.5707963050])
            u_ = ts(tabs, -1.0, ALU.mult, 1.0, ALU.add)
            u_ = ts(u_, 1e-30, ALU.add)
            sq1mt = act(act(u_, AF.Ln), AF.Exp, scale=0.5)
            acos_t = tt(poly, sq1mt, ALU.mult)
            ind = ts(r_, 0.0, ALU.is_ge)
            sgn = ts(ind, 2.0, ALU.mult, -1.0, ALU.add)
            pio = ts(ind, -math.pi, ALU.mult, math.pi, ALU.add)
            acos_r = tt(tt(acos_t, sgn, ALU.mult), pio, ALU.add)
            phi = ts(acos_r, 1.0 / 3.0, ALU.mult)

            # cos/sin Taylor on [0, pi/3]; cos(phi+2pi/3) = -.5 c - (v3/2) s
            z = tt(phi, phi, ALU.mult)
            cosp = poly_eval(z, [1.0 / 40320, -1.0 / 720, 1.0 / 24, -0.5, 1.0])
            sinp = poly_eval(z, [-1.0 / 5040, 1.0 / 120, -1.0 / 6, 1.0])
            sinp = tt(sinp, phi, ALU.mult)
            halfc = ts(cosp, -0.5, ALU.mult)
            cosp2 = stt(sinp, -math.sqrt(3.0) / 2.0, halfc, ALU.mult, ALU.add)

            twop = ts(p_, 2.0, ALU.mult)
            eigs = new([P, 3, Tn])
            e1t = tt(twop, cosp, ALU.mult)
            nc.vector.tensor_tensor(out=eigs[:, 0], in0=e1t, in1=q, op=ALU.add)
            e3t = tt(twop, cosp2, ALU.mult)
            nc.vector.tensor_tensor(out=eigs[:, 2], in0=e3t, in1=q, op=ALU.add)
            q3 = ts(q, 3.0, ALU.mult)
            e12 = tt(eigs[:, 0], eigs[:, 2], ALU.add)
            nc.vector.tensor_tensor(out=eigs[:, 1], in0=q3, in1=e12,
                                    op=ALU.subtract)

            eig_c = ts(eigs.rearrange("p k t -> p (k t)"), 0.0, ALU.max,
                       1e-30, ALU.add)                                # [P,3Tn]
            sv = act(act(eig_c, AF.Ln), AF.Exp, scale=0.5)
            sv = sv.rearrange("p (k t) -> p k t", k=3)

            dind = ts(detR, 0.0, ALU.is_ge)
            dsgn = ts(dind, 2.0, ALU.mult, -1.0, ALU.add)
            s12 = tt(sv[:, 0], sv[:, 1], ALU.add)
            ds3 = tt(dsgn, sv[:, 2], ALU.mult)
            trace = tt(s12, ds3, ALU.add)                             # [P,Tn]

            e_ = stt(trace, -2.0, e0, ALU.mult, ALU.add)
            e_ = ts(e_, 0.0, ALU.max)
            arg = tt(e_, invn_t, ALU.mult)
            arg = ts(arg, 1e-7, ALU.add)
            y0 = act(act(arg, AF.Ln), AF.Exp, scale=0.5)
            ry = recip(y0)
            ay = tt(arg, ry, ALU.mult)
            outv = ts(tt(y0, ay, ALU.add), 0.5, ALU.mult)

            nc.sync.dma_start(out=outd[:], in_=outv)

    nc.compile()

    # collapse redundant ACT table loads: every function we use (Copy,
    # Square, Ln, Exp) lives in natural_log_exp_and_others, but the
    # chooser ping-pongs between smaller sets. Retarget all loads to the
    # combined set and drop the now-redundant ones (keeping any that
    # carry sync commands).
    tables = list(get_activation_tables(nc.m.arch).keys())
    target = tables.index("natural_log_exp_and_others")
    for blk in nc.main_func.blocks:
        seen = False
        drop = []
        for inst in list(blk.instructions):
            if isinstance(inst, mybir.InstLoadActFuncSet):
                inst.act_func_set_id = target
                si = inst.sync_info
                has_sync = si is not None and (si.on_wait or si.on_update)
                if seen and not has_sync:
                    drop.append(inst)
                    continue
                seen = True
        for inst in drop:
            blk.instructions.remove(inst)
    return nc


def get_nc(n_tiles=T):
    if n_tiles not in _CACHE:
        _CACHE[n_tiles] = _build(n_tiles)
    return _CACHE[n_tiles]


def _planarize(A):
    """[B, (a c)] -> [B, (c a)] rows."""
    B = A.shape[0]
    return np.ascontiguousarray(
        A.reshape(B, M, 3).transpose(0, 2, 1).reshape(B, D))


def _prep_core_inputs(X, Y, nf, n_tiles):
    invn = (np.float32(1.0) / nf).astype(np.float32)
    consts = np.empty((P, D + 2 * n_tiles), np.float32)
    consts[:, 0:D] = np.tile(np.arange(M, dtype=np.float32), 3)[None, :]
    consts[:, D:D + n_tiles] = nf.reshape(n_tiles, P).T
    consts[:, D + n_tiles:] = invn.reshape(n_tiles, P).T
    return {
        "x": _planarize(X),
        "y": _planarize(Y),
        "consts": consts,
    }


def kernel(input, target, num_atoms):
    from concourse.bass_utils import run_bass_kernel_spmd

    X = np.asarray(input, dtype=np.float32)
    Y = np.asarray(target, dtype=np.float32)
    nf = np.asarray(num_atoms).astype(np.float32)
    B = X.shape[0]
    assert B == NCORES * S, f"unexpected batch {B}"

    nc = get_nc(T)
    in_maps = []
    for c in range(NCORES):
        sl = slice(c * S, (c + 1) * S)
        in_maps.append(_prep_core_inputs(X[sl], Y[sl], nf[sl], T))
    res = run_bass_kernel_spmd(nc, in_maps, list(range(NCORES))).results
    out = np.empty((NCORES, S), np.float32)
    for c in range(NCORES):
        out[c] = res[c]["out"].T.reshape(S)   # out[p,t] -> sample t*P+p
    return out.reshape(B)



# revision 4
# speedup vs baseline: 2.3645x; 2.3645x over previous
"""Trainium2 Bass kernel for nn_Coords2RMSD (masked Kabsch RMSD loss).

Pure data parallel over 8 NeuronCores (1024 samples each). Host prepares
ATOM-MAJOR bf16 tensors per core: x[atom, coord, sample] so the atom axis
lands on SBUF partitions in chunks of 128. Per chunk the DVE applies the
prefix mask with tensor_paged_mask (bf16 2x mode), computes the 9 per-sample
correlation products as 2x-mode tensor_tensor ops, ACT squares the masked
tensors, and the Tensor engine reduces every stream over the atom axis with
one-hot-column stationary matmuls that accumulate all 16 per-sample
quantities into a dense [16, 1024] PSUM block (R_ij x9, sx x3, sy x3,
ssx+ssy). A PE transpose turns the quantities sample-major and a closed-form
3x3 eigenvalue epilogue (trig method, ported from the sample-major kernel)
turns them into the RMSD.
"""
import math
import numpy as np

P = 128          # partitions
M = 768          # max atoms
NCORES = 8
T = 8            # column blocks of 128 samples (epilogue free dim)
S = P * T        # samples per core = 1024
NCH = M // P     # atom chunks = 6
NQ = 16          # quantities: 9 R_ij, 3 sx, 3 sy, 1 ssx+ssy
D3 = 3 * S       # chunk tile free size = 3072

_CACHE = {}


def _build():
    import concourse.bacc as bacc
    import concourse.mybir as mybir
    from concourse.tile import TileContext
    from concourse.hw_specs import get_activation_tables

    f32 = mybir.dt.float32
    bf16 = mybir.dt.bfloat16
    f16 = mybir.dt.float16
    ALU = mybir.AluOpType
    AF = mybir.ActivationFunctionType

    nc = bacc.Bacc()
    xd = nc.declare_dram_parameter("x", [M, D3], bf16, isOutput=False)
    yd = nc.declare_dram_parameter("y", [M, D3], bf16, isOutput=False)
    nbd = nc.declare_dram_parameter("nb", [P, S], f16, isOutput=False)
    cd = nc.declare_dram_parameter("consts", [P, NCH + T], f32, isOutput=False)
    wd = nc.declare_dram_parameter("w", [P, NQ * NQ], bf16, isOutput=False)
    idd = nc.declare_dram_parameter("ident", [NQ, NQ], f32, isOutput=False)
    outd = nc.declare_dram_parameter("out", [P, T], f32, isOutput=True)

    with TileContext(nc) as tc:
        with tc.tile_pool(name="io", bufs=3) as io, \
             tc.tile_pool(name="wk", bufs=2) as wk, \
             tc.tile_pool(name="ps", bufs=1, space="PSUM") as ps, \
             tc.tile_pool(name="pt", bufs=2, space="PSUM") as ptp, \
             tc.tile_pool(name="st", bufs=1) as st:
            nb_t = st.tile([P, S], f16)
            nc.sync.dma_start(out=nb_t[:], in_=nbd[:])
            c_t = st.tile([P, NCH + T], f32)
            nc.sync.dma_start(out=c_t[:], in_=cd[:])
            w_t = st.tile([P, NQ * NQ], bf16)
            nc.sync.dma_start(out=w_t[:], in_=wd[:])
            id_t = st.tile([NQ, NQ], f32)
            nc.sync.dma_start(out=id_t[:], in_=idd[:])
            invn_t = c_t[:, NCH:NCH + T]

            pacc = ps.tile([NQ, S], f32)   # [16 quantities, 1024 samples]

            started = [False, False]
            for c in range(NCH):
                sl = slice(c * P, (c + 1) * P)
                xt = io.tile([P, D3], bf16, tag="x")
                nc.sync.dma_start(out=xt[:], in_=xd[sl, :])
                yt = io.tile([P, D3], bf16, tag="y")
                nc.sync.dma_start(out=yt[:], in_=yd[sl, :])
                xt3 = xt[:].rearrange("p (i s) -> p i s", i=3)
                yt3 = yt[:].rearrange("p (i s) -> p i s", i=3)

                # prefix mask (atom index < n): TS 4x mode, then TT mults
                msk = wk.tile([P, S], bf16, tag="msk")
                nc.vector.tensor_scalar(
                    out=msk[:], in0=nb_t[:], scalar1=c_t[:, c:c + 1],
                    scalar2=None, op0=ALU.is_gt)
                mskb = msk[:].unsqueeze(1).broadcast_to([P, 3, S])
                xm = wk.tile([P, D3], bf16, tag="xm")
                xm3 = xm[:].rearrange("p (i s) -> p i s", i=3)
                nc.vector.tensor_tensor(out=xm3, in0=xt3, in1=mskb,
                                        op=ALU.mult)
                ym = wk.tile([P, D3], bf16, tag="ym")
                ym3 = ym[:].rearrange("p (i s) -> p i s", i=3)
                nc.vector.tensor_tensor(out=ym3, in0=yt3, in1=mskb,
                                        op=ALU.mult)

                # masked squares on ACT
                xsq = wk.tile([P, D3], bf16, tag="xsq")
                nc.scalar.activation(out=xsq[:], in_=xm[:], func=AF.Square)
                ysq = wk.tile([P, D3], bf16, tag="ysq")
                nc.scalar.activation(out=ysq[:], in_=ym[:], func=AF.Square)
                xsq3 = xsq[:].rearrange("p (i s) -> p i s", i=3)
                ysq3 = ysq[:].rearrange("p (i s) -> p i s", i=3)

                # products p_i[j, s] = xm_i * ym_j  (DVE 2x bf16)
                pr3 = []
                for i in range(3):
                    p_i = wk.tile([P, D3], bf16, tag=f"p{i}")
                    p_i3 = p_i[:].rearrange("p (i s) -> p i s", i=3)
                    nc.vector.tensor_tensor(
                        out=p_i3, in0=xm3[:, i:i + 1, :].broadcast_to([P, 3, S]),
                        in1=ym3, op=ALU.mult)
                    pr3.append(p_i3)

                # reduce every stream over atoms with one-hot stationary
                # matmuls; all 16 quantity rows accumulate in one PSUM block
                seq = []
                for i in range(3):
                    seq.append((9 + i, xm3[:, i, :]))
                for j in range(3):
                    seq.append((12 + j, ym3[:, j, :]))
                for i in range(3):
                    seq.append((15, xsq3[:, i, :]))
                    seq.append((15, ysq3[:, i, :]))
                for i in range(3):
                    for j in range(3):
                        seq.append((3 * i + j, pr3[i][:, j, :]))
                last = c == NCH - 1
                for qi, (q, rhs) in enumerate(seq):
                    for h in range(2):
                        hs = slice(h * 512, (h + 1) * 512)
                        nc.tensor.matmul(
                            pacc[:, hs], w_t[:, NQ * q:NQ * (q + 1)], rhs[:, hs],
                            start=not started[h],
                            stop=last and qi == len(seq) - 1,
                            skip_group_check=True)
                        started[h] = True

            # extract quantities, transpose to sample-major [128, 16*8]
            qs = st.tile([NQ, S], f32)
            nc.vector.tensor_copy(qs[:], pacc[:])
            epi = st.tile([P, NQ * T], f32)
            epi3 = epi[:].rearrange("p (q t) -> p q t", q=NQ)
            for k in range(T):
                tp = ptp.tile([P, NQ], f32, tag="tp")
                nc.tensor.transpose(tp[:], qs[:, k * P:(k + 1) * P], id_t[:])
                nc.vector.tensor_copy(epi3[:, :, k], tp[:])

            # ---------------- epilogue (batched over [P, ..., T]) ----------
            Tn = T
            cnt = [0]

            def new(shape):
                cnt[0] += 1
                free = int(np.prod(shape[1:]))
                r = st.tile([P, free], f32, tag=f"e{cnt[0]}")
                ap = r[:]
                if len(shape) > 2:
                    names = " ".join(f"d{i}" for i in range(len(shape) - 1))
                    ap = ap.rearrange(f"p ({names}) -> p {names}",
                                      **{f"d{i}": int(shape[1 + i])
                                         for i in range(len(shape) - 1)})
                return ap

            def tt(a, b, op, shape=None):
                r = new(list(shape or a.shape))
                nc.vector.tensor_tensor(out=r, in0=a, in1=b, op=op)
                return r

            def ts(a, s1, op0, s2=None, op1=None):
                r = new(list(a.shape))
                if op1 is None:
                    nc.vector.tensor_scalar(out=r, in0=a, scalar1=s1,
                                            scalar2=None, op0=op0)
                else:
                    nc.vector.tensor_scalar(out=r, in0=a, scalar1=s1,
                                            scalar2=s2, op0=op0, op1=op1)
                return r

            def stt(a, s, b, op0, op1):
                r = new(list(a.shape))
                nc.vector.scalar_tensor_tensor(out=r, in0=a, scalar=s,
                                               in1=b, op0=op0, op1=op1)
                return r

            def act(a, func, scale=1.0, bias=0.0):
                r = new(list(a.shape))
                nc.scalar.activation(out=r, in_=a, func=func,
                                     scale=scale, bias=bias)
                return r

            def recip(a):
                r = new(list(a.shape))
                nc.vector.reciprocal(out=r, in_=a)
                return r

            def red_inner(a, n_keep):
                r = new([P, n_keep])
                nc.vector.tensor_reduce(out=r, in_=a,
                                        axis=mybir.AxisListType.X, op=ALU.add)
                return r

            def poly_eval(x, coeffs):
                g = ts(x, coeffs[0], ALU.mult)
                for c in coeffs[1:-1]:
                    g = stt(g, c, x, ALU.add, ALU.mult)
                return ts(g, coeffs[-1], ALU.add)

            mmv = epi[:, 0:9 * Tn].rearrange("p (i j t) -> p i j t", i=3, j=3)
            sxv = epi[:, 9 * Tn:12 * Tn].rearrange("p (i t) -> p i t", i=3)
            syv = epi[:, 12 * Tn:15 * Tn].rearrange("p (i t) -> p i t", i=3)
            ss = epi[:, 15 * Tn:16 * Tn]          # ssx + ssy, [P, Tn]
            invn_b3 = invn_t.unsqueeze(1).broadcast_to([P, 3, Tn])

            # R_ij = m_ij - (sx_i * invn) * sy_j
            meanx = tt(sxv, invn_b3, ALU.mult)                       # [P,3,Tn]
            meanx_v = meanx.unsqueeze(2).broadcast_to([P, 3, 3, Tn])
            sy_v = syv.unsqueeze(1).broadcast_to([P, 3, 3, Tn])
            mxsy = tt(meanx_v, sy_v, ALU.mult)
            Rv = tt(mmv, mxsy, ALU.subtract)                         # [P,3,3,Tn]

            # e0 = ssx + ssy - (|sx|^2 + |sy|^2) * invn
            sx2 = tt(sxv, sxv, ALU.mult)
            sy2 = tt(syv, syv, ALU.mult)
            nrm = tt(sx2, sy2, ALU.add)
            nrms = red_inner(nrm.rearrange("p i t -> p t i"), Tn)
            nrmi = tt(nrms, invn_t, ALU.mult)
            e0 = tt(ss, nrmi, ALU.subtract)                          # [P,Tn]

            # A = R^T R (batched outer products over k)
            Av = new([P, 3, 3, Tn])
            for k in range(3):
                rk = Rv[:, k]
                rk_a = rk.unsqueeze(2).broadcast_to([P, 3, 3, Tn])
                rk_b = rk.unsqueeze(1).broadcast_to([P, 3, 3, Tn])
                if k == 0:
                    nc.vector.tensor_tensor(out=Av, in0=rk_a, in1=rk_b,
                                            op=ALU.mult)
                else:
                    pk = tt(rk_a, rk_b, ALU.mult)
                    nc.vector.tensor_tensor(out=Av, in0=Av, in1=pk, op=ALU.add)
            Aflat = Av.rearrange("p a b t -> p (a b) t")
            Adiag = Aflat[:, ::4]                                    # [P,3,Tn]

            q = ts(red_inner(Adiag.rearrange("p a t -> p t a"), Tn),
                   1.0 / 3.0, ALU.mult)                              # [P,Tn]
            q_b3 = q.unsqueeze(1).broadcast_to([P, 3, Tn])
            bdiag = tt(Adiag, q_b3, ALU.subtract)

            # p2 = sum(bdiag^2) + (sum(A^2) - sum(diag(A)^2))
            asq = tt(Aflat, Aflat, ALU.mult)
            allsq = red_inner(asq.rearrange("p a t -> p t a"), Tn)
            dsq = tt(Adiag, Adiag, ALU.mult)
            dsqs = red_inner(dsq.rearrange("p a t -> p t a"), Tn)
            bsq = tt(bdiag, bdiag, ALU.mult)
            bsqs = red_inner(bsq.rearrange("p a t -> p t a"), Tn)
            offs2 = tt(allsq, dsqs, ALU.subtract)
            p2 = tt(bsqs, offs2, ALU.add)                            # [P,Tn]

            # log-space: p = (p2/6)^0.5 and invp^3 = (p2/6)^-1.5
            p2e = ts(p2, 1e-10, ALU.add)
            lnp2 = act(p2e, AF.Ln, scale=1.0 / 6.0)
            p_ = act(lnp2, AF.Exp, scale=0.5)
            ip3 = act(lnp2, AF.Exp, scale=-1.5)

            # batched determinants of W0=R and W1=B (= A - q I)
            Dw = new([P, 2, 3, 3, Tn])
            nc.vector.tensor_copy(Dw[:, 0], Rv)
            nc.vector.tensor_copy(Dw[:, 1], Av)
            Dw_diag = Dw.rearrange("p w a b t -> p w (a b) t")[:, 1, ::4]
            nc.vector.tensor_tensor(out=Dw_diag, in0=Adiag, in1=q_b3,
                                    op=ALU.subtract)

            def dsl(i, j):
                return Dw[:, :, i, j]                                # [P,2,Tn]

            u1 = tt(dsl(1, 1), dsl(2, 2), ALU.mult)
            u2 = tt(dsl(1, 2), dsl(2, 1), ALU.mult)
            cof0 = tt(dsl(0, 0), tt(u1, u2, ALU.subtract), ALU.mult)
            u3 = tt(dsl(1, 0), dsl(2, 2), ALU.mult)
            u4 = tt(dsl(1, 2), dsl(2, 0), ALU.mult)
            cof1 = tt(dsl(0, 1), tt(u3, u4, ALU.subtract), ALU.mult)
            u5 = tt(dsl(1, 0), dsl(2, 1), ALU.mult)
            u6 = tt(dsl(1, 1), dsl(2, 0), ALU.mult)
            cof2 = tt(dsl(0, 2), tt(u5, u6, ALU.subtract), ALU.mult)
            dets = tt(tt(cof0, cof1, ALU.subtract), cof2, ALU.add)   # [P,2,Tn]
            detR = dets[:, 0]
            detB = dets[:, 1]

            # r = clamp(0.5 * detB * invp^3, -1, 1)
            rr = tt(detB, ip3, ALU.mult)
            r_ = ts(rr, 0.5, ALU.mult, 1.0, ALU.min)
            r_ = ts(r_, -1.0, ALU.max)

            # acos(r)/3 via |r| polynomial (A&S 4.4.46) + reflection
            rneg = ts(r_, -1.0, ALU.mult)
            tabs = tt(r_, rneg, ALU.max)
            poly = poly_eval(tabs, [-0.0012624911, 0.0066700901, -0.0170881256,
                                    0.0308918810, -0.0501743046, 0.0889789874,
                                    -0.2145988016, 1.5707963050])
            u_ = ts(tabs, -1.0, ALU.mult, 1.0, ALU.add)
            u_ = ts(u_, 1e-30, ALU.add)
            sq1mt = act(act(u_, AF.Ln), AF.Exp, scale=0.5)
            acos_t = tt(poly, sq1mt, ALU.mult)
            ind = ts(r_, 0.0, ALU.is_ge)
            sgn = ts(ind, 2.0, ALU.mult, -1.0, ALU.add)
            pio = ts(ind, -math.pi, ALU.mult, math.pi, ALU.add)
            acos_r = tt(tt(acos_t, sgn, ALU.mult), pio, ALU.add)
            phi = ts(acos_r, 1.0 / 3.0, ALU.mult)

            # cos/sin Taylor on [0, pi/3]; cos(phi+2pi/3) = -.5 c - (v3/2) s
            z = tt(phi, phi, ALU.mult)
            cosp = poly_eval(z, [1.0 / 40320, -1.0 / 720, 1.0 / 24, -0.5, 1.0])
            sinp = poly_eval(z, [-1.0 / 5040, 1.0 / 120, -1.0 / 6, 1.0])
            sinp = tt(sinp, phi, ALU.mult)
            halfc = ts(cosp, -0.5, ALU.mult)
            cosp2 = stt(sinp, -math.sqrt(3.0) / 2.0, halfc, ALU.mult, ALU.add)

            twop = ts(p_, 2.0, ALU.mult)
            eigs = new([P, 3, Tn])
            e1t = tt(twop, cosp, ALU.mult)
            nc.vector.tensor_tensor(out=eigs[:, 0], in0=e1t, in1=q, op=ALU.add)
            e3t = tt(twop, cosp2, ALU.mult)
            nc.vector.tensor_tensor(out=eigs[:, 2], in0=e3t, in1=q, op=ALU.add)
            q3 = ts(q, 3.0, ALU.mult)
            e12 = tt(eigs[:, 0], eigs[:, 2], ALU.add)
            nc.vector.tensor_tensor(out=eigs[:, 1], in0=q3, in1=e12,
                                    op=ALU.subtract)

            eig_c = ts(eigs.rearrange("p k t -> p (k t)"), 0.0, ALU.max,
                       1e-30, ALU.add)                                # [P,3Tn]
            sv = act(act(eig_c, AF.Ln), AF.Exp, scale=0.5)
            sv = sv.rearrange("p (k t) -> p k t", k=3)

            dind = ts(detR, 0.0, ALU.is_ge)
            dsgn = ts(dind, 2.0, ALU.mult, -1.0, ALU.add)
            s12 = tt(sv[:, 0], sv[:, 1], ALU.add)
            ds3 = tt(dsgn, sv[:, 2], ALU.mult)
            trace = tt(s12, ds3, ALU.add)                             # [P,Tn]

            e_ = stt(trace, -2.0, e0, ALU.mult, ALU.add)
            e_ = ts(e_, 0.0, ALU.max)
            arg = tt(e_, invn_t, ALU.mult)
            arg = ts(arg, 1e-7, ALU.add)
            y0 = act(act(arg, AF.Ln), AF.Exp, scale=0.5)
            ry = recip(y0)
            ay = tt(arg, ry, ALU.mult)
            outv = ts(tt(y0, ay, ALU.add), 0.5, ALU.mult)

            nc.sync.dma_start(out=outd[:], in_=outv)

    nc.compile()

    # collapse redundant ACT table loads (all funcs used live in
    # natural_log_exp_and_others)
    tables = list(get_activation_tables(nc.m.arch).keys())
    target = tables.index("natural_log_exp_and_others")
    for blk in nc.main_func.blocks:
        seen = False
        drop = []
        for inst in list(blk.instructions):
            if isinstance(inst, mybir.InstLoadActFuncSet):
                inst.act_func_set_id = target
                si = inst.sync_info
                has_sync = si is not None and (si.on_wait or si.on_update)
                if seen and not has_sync:
                    drop.append(inst)
                    continue
                seen = True
        for inst in drop:
            blk.instructions.remove(inst)
    return nc


def get_nc(n_tiles=T):
    if "nc" not in _CACHE:
        _CACHE["nc"] = _build()
    return _CACHE["nc"]


def _prep_core_inputs(X, Y, nf, n_tiles=T):
    import ml_dtypes
    bf = ml_dtypes.bfloat16
    xT = np.ascontiguousarray(
        X.reshape(S, M, 3).transpose(1, 2, 0).reshape(M, D3)).astype(bf)
    yT = np.ascontiguousarray(
        Y.reshape(S, M, 3).transpose(1, 2, 0).reshape(M, D3)).astype(bf)
    nb = np.repeat(nf[None, :].astype(np.float16), P, axis=0)
    consts = np.empty((P, NCH + T), np.float32)
    for c in range(NCH):
        consts[:, c] = c * P + np.arange(P, dtype=np.float32)
    consts[:, NCH:] = (np.float32(1.0) / nf).astype(np.float32).reshape(T, P).T
    w = np.tile(np.eye(NQ, dtype=np.float32).reshape(-1), (P, 1)).astype(bf)
    ident = np.eye(NQ, dtype=np.float32)
    return {"x": xT, "y": yT, "nb": nb, "consts": consts, "w": w,
            "ident": ident}


def kernel(input, target, num_atoms):
    from concourse.bass_utils import run_bass_kernel_spmd

    X = np.asarray(input, dtype=np.float32)
    Y = np.asarray(target, dtype=np.float32)
    nf = np.asarray(num_atoms).astype(np.float32)
    B = X.shape[0]
    assert B == NCORES * S, f"unexpected batch {B}"

    nc = get_nc()
    in_maps = []
    for c in range(NCORES):
        sl = slice(c * S, (c + 1) * S)
        in_maps.append(_prep_core_inputs(X[sl], Y[sl], nf[sl]))
    res = run_bass_kernel_spmd(nc, in_maps, list(range(NCORES))).results
    out = np.empty((NCORES, S), np.float32)
    for c in range(NCORES):
        out[c] = res[c]["out"].T.reshape(S)   # out[p,t] -> sample t*P+p
    return out.reshape(B)
